# revision 5
# baseline (speedup 1.0000x reference)
"""MLAttention (label-pooling attention) Trainium2 Bass kernel.

Computes, for full inputs:
    scores = einsum('bsh,lh->bls', inputs, W)
    scores = where(mask==0, -inf, scores)
    attn   = softmax(scores, axis=-1)
    out    = einsum('bls,bsh->blh', attn, inputs)

Label-parallel across 8 NeuronCores: L=28415 padded to 28672 = 8*3584.
Each core gets its own W shard [3584, 512]; inputs/masks replicated.
Host concatenates the 8 per-core outputs [B, 3584, H] and trims to L.

Transpose-free dataflow (variant c). Scores are computed TRANSPOSED,
in [s, l] layout, so the exp() tile is directly the stationary operand
of the second matmul -- no PE transposes, no PSUM->SBUF transpose copy.
The softmax mask is folded into the exp bias (per-partition = per-s).
Row-sums (softmax denominators) come from N=1 matmuls against a ones
column that reuse mm2's already-loaded stationary, accumulating into a
separate PSUM bank; normalization happens in the final ACT copy via a
per-partition reciprocal scale.

All matmul operands are bf16 (host-side cast): full-rate PE streaming
plus fast weight load, half the input DMA bytes. Accumulation is fp32
in PSUM; exp() runs on ACT in fp32 from PSUM and rounds to bf16.

Per-core steady state, per (b, 512-label group) step:
  PE  : mm1 = 16 MMs N=512   scoresT chunks [s128, 512] x4, acc over h
        mm2 = 16 MMs N=512 + 16 tiny N=1 (rowsums), per 128-label tile
  ACT : 4x exp (PSUM->SBUF bf16, mask bias), 4x scaled out copy
  DVE : 4x reciprocal [128,1]
  DMA : x/xt/W-group prefetch in, 4x out tile [128,512] f32 out
with a one-step software pipeline: group g's mm1 is emitted before
group g-1's mm2, so the in-order PE queue always has independent work
while g's exp chain completes on ACT.
"""

from contextlib import ExitStack

import ml_dtypes
import numpy as np

import concourse.bass as bass
import concourse.mybir as mybir
import concourse.tile as tile
from concourse import bacc, bass_utils
from concourse.bass import ds, ts

F32 = mybir.dt.float32
BF16 = mybir.dt.bfloat16

# Problem shapes (hardcoded per contract).
B, S, H, L = 4, 512, 512, 28415
N_CORES = 8
LSH = 3584               # per-core padded label count (28 tiles of 128)
L_PAD = LSH * N_CORES    # 28672


def build_module_c(b_sz=B, s_sz=S, h_sz=H, lsh=LSH, n_devices=N_CORES):
    P = 128
    KH = h_sz // P   # H contraction chunks (mm1)
    KS = s_sz // P   # S contraction chunks (mm2) == score s-tiles
    LG = 512         # label group per step
    NG = lsh // LG   # groups per batch
    NSUB = LG // P   # 128-label tiles per group

    nc = bacc.Bacc(
        "TRN2", target_bir_lowering=False, debug=False, num_devices=n_devices
    )
    x_d = nc.dram_tensor("x", [b_sz, s_sz, h_sz], BF16, kind="ExternalInput").ap()
    xt_d = nc.dram_tensor("xt", [b_sz, h_sz, s_sz], BF16, kind="ExternalInput").ap()
    wt_d = nc.dram_tensor("wt", [h_sz, lsh], BF16, kind="ExternalInput").ap()
    m_d = nc.dram_tensor("m", [b_sz, s_sz], F32, kind="ExternalInput").ap()
    o_d = nc.dram_tensor("o", [b_sz, lsh, h_sz], F32, kind="ExternalOutput").ap()

    with tile.TileContext(nc) as tc, ExitStack() as ctx:
        const = ctx.enter_context(tc.tile_pool(name="const", bufs=1))
        res = ctx.enter_context(tc.tile_pool(name="res", bufs=1))
        work = ctx.enter_context(tc.tile_pool(name="work", bufs=3))
        psum = ctx.enter_context(tc.tile_pool(name="psum", bufs=2, space="PSUM"))

        ones_f = const.tile([P, 1], F32)
        nc.gpsimd.memset(ones_f[:], 1.0)
        ones_col = const.tile([P, 1], BF16)
        nc.vector.tensor_copy(ones_col[:], ones_f[:])

        # Resident SBUF tensors (bf16 straight from DMA, no casts).
        XT = res.tile([P, b_sz, KH, s_sz], BF16)  # XT[h%128, b, h//128, s]
        XB = res.tile([P, b_sz, KS, h_sz], BF16)  # XB[s%128, b, s//128, h]
        WT = res.tile([P, KH, lsh], BF16)         # WT[h%128, h//128, l]
        MB = res.tile([P, b_sz, KS], F32)         # exp bias: (mask-1)*30 per s

        def mask_setup():
            mbr = work.tile([P, b_sz, KS], F32, tag="mbr")
            nc.sync.dma_start(mbr[:], m_d.rearrange("b (c p) -> p b c", p=P))
            nc.vector.tensor_scalar_mul(out=mbr[:], in0=mbr[:], scalar1=30.0)
            nc.vector.tensor_scalar_add(out=MB[:], in0=mbr[:], scalar1=-30.0)

        def xt_setup(b):
            nc.sync.dma_start(
                XT[:, b], xt_d[b].rearrange("(k p) s -> p k s", p=P)
            )

        def xb_setup(b):
            nc.sync.dma_start(
                XB[:, b], x_d[b].rearrange("(c p) h -> p c h", p=P)
            )

        def w_setup(g):
            nc.sync.dma_start(
                WT[:, :, ts(g, LG)],
                wt_d[:, ts(g, LG)].rearrange("(k p) l -> p k l", p=P),
            )

        def front(b, g):
            """mm1 (scoresT chunks) + exp for group (b, g)."""
            ps_sct = psum.tile([P, KS, LG], F32, tag="ps_sct", bufs=1)
            exp_g = work.tile([P, KS, LG], BF16, tag="exp", bufs=2)
            for st in range(KS):
                for k in range(KH):
                    nc.tensor.matmul(
                        ps_sct[:, st, :],
                        XT[:, b, k, ts(st, P)],
                        WT[:, k, ts(g, LG)],
                        start=(k == 0),
                        stop=(k == KH - 1),
                    )
                nc.scalar.activation(
                    exp_g[:, st, :], ps_sct[:, st, :],
                    mybir.ActivationFunctionType.Exp,
                    bias=MB[:, b, st : st + 1],
                )
            return exp_g

        def back(b, g, exp_g):
            """mm2 + rowsums + normalize + store for group (b, g).

            The normalize copy runs on DVE (tensor_scalar mul with a
            per-partition reciprocal), keeping ACT exp-only so exp(g)
            never queues behind copies -- that WAR chain was the main
            steady-state PE bubble."""
            ps_sums = psum.tile([P, NSUB], F32, tag="ps_sums", bufs=2)
            recips = work.tile([P, NSUB], F32, tag="recips", bufs=4)
            for lt in range(NSUB):
                ps_out = psum.tile([P, h_sz], F32, tag="ps_out", bufs=2)
                for sc in range(KS):
                    stat = exp_g[:, sc, ts(lt, P)]
                    nc.tensor.matmul(
                        ps_out[:], stat, XB[:, b, sc, :],
                        start=(sc == 0), stop=(sc == KS - 1),
                    )
                    nc.tensor.matmul(
                        ps_sums[:, lt : lt + 1], stat, ones_col[:],
                        start=(sc == 0), stop=(sc == KS - 1),
                    )
                nc.vector.reciprocal(
                    recips[:, lt : lt + 1], ps_sums[:, lt : lt + 1]
                )
                out_t = work.tile([P, h_sz], F32, tag="out", bufs=3)
                nc.vector.tensor_scalar_mul(
                    out=out_t[:], in0=ps_out[:],
                    scalar1=recips[:, lt : lt + 1],
                )
                nc.sync.dma_start(o_d[b, ds(g * LG + lt * P, P), :], out_t[:])

        # ---- emission: one-step software pipeline over (b, g) steps, with
        # W groups streaming two ahead during b=0 and b=1..3 inputs
        # prefetched early in the prior batch's pass.
        # DMA queue order matters for fill: front(0,0) needs mask + w0 +
        # all of XT[0]; XB[0] is not read until back(0,0), one step later.
        mask_setup()
        w_setup(0)
        xt_setup(0)
        w_setup(1)
        xb_setup(0)

        pend = [None]
        for b in range(b_sz):
            for g in range(NG):
                if b == 0 and g + 2 < NG:
                    w_setup(g + 2)
                exp_g = front(b, g)
                if b < b_sz - 1 and g == 0:
                    xt_setup(b + 1)
                    xb_setup(b + 1)
                if pend[0] is not None:
                    back(*pend[0])
                pend[0] = (b, g, exp_g)
        back(*pend[0])

    nc.compile()
    return nc


_CACHE = {}

VARIANT = "c"


def _get_module():
    if VARIANT not in _CACHE:
        _CACHE[VARIANT] = build_module_c()
    return _CACHE[VARIANT]


def _run(inputs: np.ndarray, masks: np.ndarray, W: np.ndarray, **spmd_kwargs):
    """Run on 8 cores; returns (full output, BassKernelResults)."""
    nc = _get_module()

    x32 = np.ascontiguousarray(inputs, dtype=np.float32)
    x = x32.astype(ml_dtypes.bfloat16)
    xt = np.ascontiguousarray(np.swapaxes(x32, 1, 2)).astype(ml_dtypes.bfloat16)
    mf = np.ascontiguousarray(masks, dtype=np.float32)
    wt_pad = np.zeros((H, L_PAD), dtype=np.float32)
    wt_pad[:, :L] = W.T
    wt_bf = wt_pad.astype(ml_dtypes.bfloat16)

    in_maps = [
        {
            "x": x,
            "xt": xt,
            "m": mf,
            "wt": np.ascontiguousarray(wt_bf[:, c * LSH : (c + 1) * LSH]),
        }
        for c in range(N_CORES)
    ]
    res = bass_utils.run_bass_kernel_spmd(
        nc, in_maps, core_ids=list(range(N_CORES)), **spmd_kwargs
    )
    out = np.concatenate([res.results[c]["o"] for c in range(N_CORES)], axis=1)
    return np.ascontiguousarray(out[:, :L, :]), res


def kernel(inputs: np.ndarray, masks: np.ndarray, W: np.ndarray) -> np.ndarray:
    out, _ = _run(inputs, masks, W)
    return out


# revision 6
# speedup vs baseline: 1.4644x; 1.4644x over previous
"""MLAttention (label-pooling attention) Trainium2 Bass kernel.

Computes, for full inputs:
    scores = einsum('bsh,lh->bls', inputs, W)
    scores = where(mask==0, -inf, scores)
    attn   = softmax(scores, axis=-1)
    out    = einsum('bls,bsh->blh', attn, inputs)

Label-parallel across 8 NeuronCores: L=28415 padded to 28672 = 8*3584.
Each core gets its own W shard [3584, 512]; inputs/masks replicated.
Host concatenates the 8 per-core outputs [B, 3584, H] and trims to L.

Transpose-free dataflow. Scores are computed TRANSPOSED, in [s, l]
layout, so the exp() tile is directly the stationary operand of the
second matmul -- no PE transposes. The softmax mask is folded into the
exp bias (per-partition = per-s). Row-sums (softmax denominators) come
from N=1 matmuls against a ones column that reuse mm2's already-loaded
stationary, accumulating into a separate PSUM bank; normalization
happens in the final ACT copy via a per-partition reciprocal scale.

Matmul operands are host-cast: mm2 in bf16; mm1 either bf16 ("c") or
fp8e4m3 with DoubleRow perf mode ("d", 2 MACs/cell/cycle, halves mm1
stream time; W is pre-scaled by 2^14 into fp8 range and the exp
activation descales via its free affine scale). Accumulation is fp32
in PSUM; exp() runs on ACT in fp32 from PSUM and rounds to bf16.

Input DMAs are issued on the ACT HWDGE queue, W-shard and output DMAs
on the sync queue, so the two big fill-phase loads stream in parallel.
A one-step software pipeline (group g's mm1 emitted before group
g-1's mm2) keeps the in-order PE queue full while g's exp chain
completes on ACT.
"""

from contextlib import ExitStack

import ml_dtypes
import numpy as np

import concourse.bass as bass
import concourse.mybir as mybir
import concourse.tile as tile
from concourse import bacc, bass_utils
from concourse.bass import ds, ts

F32 = mybir.dt.float32
BF16 = mybir.dt.bfloat16
FP8 = mybir.dt.float8e4

# Problem shapes (hardcoded per contract).
B, S, H, L = 4, 512, 512, 28415
N_CORES = 8
LSH = 3584               # per-core padded label count (28 tiles of 128)
L_PAD = LSH * N_CORES    # 28672
W_SCALE = 2.0 ** 14      # fp8 variant: host premultiplies W, exp descales


def build_module(b_sz=B, s_sz=S, h_sz=H, lsh=LSH, n_devices=N_CORES,
                 mm1_fp8=False):
    P = 128
    KH = h_sz // P   # H contraction chunks (mm1)
    KS = s_sz // P   # S contraction chunks (mm2) == score s-tiles
    LG = 512         # label group per step
    NG = lsh // LG   # groups per batch
    NSUB = LG // P   # 128-label tiles per group
    mm1_dt = FP8 if mm1_fp8 else BF16

    nc = bacc.Bacc(
        "TRN2", target_bir_lowering=False, debug=False, num_devices=n_devices
    )
    x_d = nc.dram_tensor("x", [b_sz, s_sz, h_sz], BF16, kind="ExternalInput").ap()
    xt_d = nc.dram_tensor("xt", [b_sz, h_sz, s_sz], mm1_dt, kind="ExternalInput").ap()
    wt_d = nc.dram_tensor("wt", [h_sz, lsh], mm1_dt, kind="ExternalInput").ap()
    m_d = nc.dram_tensor("m", [b_sz, s_sz], F32, kind="ExternalInput").ap()
    o_d = nc.dram_tensor("o", [b_sz, lsh, h_sz], F32, kind="ExternalOutput").ap()

    with tile.TileContext(nc) as tc, ExitStack() as ctx:
        const = ctx.enter_context(tc.tile_pool(name="const", bufs=1))
        res = ctx.enter_context(tc.tile_pool(name="res", bufs=1))
        work = ctx.enter_context(tc.tile_pool(name="work", bufs=3))
        psum = ctx.enter_context(tc.tile_pool(name="psum", bufs=2, space="PSUM"))

        ones_f = const.tile([P, 1], F32)
        nc.gpsimd.memset(ones_f[:], 1.0)
        ones_col = const.tile([P, 1], BF16)
        nc.vector.tensor_copy(ones_col[:], ones_f[:])

        # Resident SBUF tensors (narrow dtypes straight from DMA, no casts).
        XT = res.tile([P, b_sz, KH, s_sz], mm1_dt)  # XT[h%128, b, h//128, s]
        XB = res.tile([P, b_sz, KS, h_sz], BF16)    # XB[s%128, b, s//128, h]
        WT = res.tile([P, KH, lsh], mm1_dt)         # WT[h%128, h//128, l]
        MB = res.tile([P, b_sz, KS], F32)           # exp bias: (mask-1)*30 per s

        def mask_setup():
            mbr = work.tile([P, b_sz, KS], F32, tag="mbr")
            nc.sync.dma_start(mbr[:], m_d.rearrange("b (c p) -> p b c", p=P))
            nc.vector.tensor_scalar_mul(out=mbr[:], in0=mbr[:], scalar1=30.0)
            nc.vector.tensor_scalar_add(out=MB[:], in0=mbr[:], scalar1=-30.0)

        # Big input loads go on the ACT HWDGE queue so they stream in
        # parallel with the W-shard loads on the sync queue.
        def xt_setup(b):
            nc.scalar.dma_start(
                XT[:, b], xt_d[b].rearrange("(k p) s -> p k s", p=P)
            )

        def xb_setup(b):
            nc.scalar.dma_start(
                XB[:, b], x_d[b].rearrange("(c p) h -> p c h", p=P)
            )

        def w_setup(g):
            nc.sync.dma_start(
                WT[:, :, ts(g, LG)],
                wt_d[:, ts(g, LG)].rearrange("(k p) l -> p k l", p=P),
            )

        exp_scale = (1.0 / W_SCALE) if mm1_fp8 else 1.0

        def front(b, g):
            """mm1 (scoresT chunks) + exp for group (b, g)."""
            ps_sct = psum.tile([P, KS, LG], F32, tag="ps_sct", bufs=1)
            exp_g = work.tile([P, KS, LG], BF16, tag="exp", bufs=2)
            for st in range(KS):
                if mm1_fp8:
                    for k2 in range(0, KH, 2):
                        nc.tensor.matmul(
                            ps_sct[:, st, :],
                            XT[:, b, ds(k2, 2), ts(st, P)],
                            WT[:, ds(k2, 2), ts(g, LG)],
                            start=(k2 == 0),
                            stop=(k2 == KH - 2),
                            perf_mode=mybir.MatmulPerfMode.DoubleRow,
                        )
                else:
                    for k in range(KH):
                        nc.tensor.matmul(
                            ps_sct[:, st, :],
                            XT[:, b, k, ts(st, P)],
                            WT[:, k, ts(g, LG)],
                            start=(k == 0),
                            stop=(k == KH - 1),
                        )
                nc.scalar.activation(
                    exp_g[:, st, :], ps_sct[:, st, :],
                    mybir.ActivationFunctionType.Exp,
                    bias=MB[:, b, st : st + 1],
                    scale=exp_scale,
                )
            return exp_g

        def back(b, g, exp_g):
            """mm2 + rowsums + normalize + store for group (b, g)."""
            ps_sums = psum.tile([P, NSUB], F32, tag="ps_sums", bufs=2)
            recips = work.tile([P, NSUB], F32, tag="recips", bufs=4)
            for lt in range(NSUB):
                ps_out = psum.tile([P, h_sz], F32, tag="ps_out", bufs=2)
                for sc in range(KS):
                    stat = exp_g[:, sc, ts(lt, P)]
                    nc.tensor.matmul(
                        ps_out[:], stat, XB[:, b, sc, :],
                        start=(sc == 0), stop=(sc == KS - 1),
                    )
                    nc.tensor.matmul(
                        ps_sums[:, lt : lt + 1], stat, ones_col[:],
                        start=(sc == 0), stop=(sc == KS - 1),
                    )
                nc.vector.reciprocal(
                    recips[:, lt : lt + 1], ps_sums[:, lt : lt + 1]
                )
                out_t = work.tile([P, h_sz], F32, tag="out", bufs=3)
                nc.scalar.activation(
                    out_t[:], ps_out[:],
                    mybir.ActivationFunctionType.Copy,
                    scale=recips[:, lt : lt + 1],
                )
                nc.sync.dma_start(o_d[b, ds(g * LG + lt * P, P), :], out_t[:])

        # ---- emission. Fill: xt0 streams on the ACT queue while mask+w0+w1
        # go on sync; xb0 follows xt0 on the ACT queue (first read is one
        # step later, in back(0,0)). W groups stream two ahead during b=0;
        # b+1 inputs prefetch early in batch b's pass.
        mask_setup()
        w_setup(0)
        xt_setup(0)
        xb_setup(0)
        w_setup(1)

        pend = [None]
        for b in range(b_sz):
            for g in range(NG):
                if b == 0 and g + 2 < NG:
                    w_setup(g + 2)
                exp_g = front(b, g)
                if b < b_sz - 1 and g == 0:
                    xt_setup(b + 1)
                    xb_setup(b + 1)
                if pend[0] is not None:
                    back(*pend[0])
                pend[0] = (b, g, exp_g)
        back(*pend[0])

    nc.compile()
    return nc


_CACHE = {}

VARIANT = "d"  # "c": bf16 mm1; "d": fp8 DoubleRow mm1


def _get_module():
    if VARIANT not in _CACHE:
        _CACHE[VARIANT] = build_module(mm1_fp8=(VARIANT == "d"))
    return _CACHE[VARIANT]


def _run(inputs: np.ndarray, masks: np.ndarray, W: np.ndarray, **spmd_kwargs):
    """Run on 8 cores; returns (full output, BassKernelResults)."""
    nc = _get_module()

    x32 = np.ascontiguousarray(inputs, dtype=np.float32)
    xt32 = np.ascontiguousarray(np.swapaxes(x32, 1, 2))
    x = x32.astype(ml_dtypes.bfloat16)
    mf = np.ascontiguousarray(masks, dtype=np.float32)
    wt_pad = np.zeros((H, L_PAD), dtype=np.float32)
    wt_pad[:, :L] = W.T
    if VARIANT == "d":
        xt = xt32.astype(ml_dtypes.float8_e4m3)
        wt_n = np.clip(wt_pad * W_SCALE, -240.0, 240.0).astype(
            ml_dtypes.float8_e4m3
        )
    else:
        xt = xt32.astype(ml_dtypes.bfloat16)
        wt_n = wt_pad.astype(ml_dtypes.bfloat16)

    in_maps = [
        {
            "x": x,
            "xt": xt,
            "m": mf,
            "wt": np.ascontiguousarray(wt_n[:, c * LSH : (c + 1) * LSH]),
        }
        for c in range(N_CORES)
    ]
    res = bass_utils.run_bass_kernel_spmd(
        nc, in_maps, core_ids=list(range(N_CORES)), **spmd_kwargs
    )
    out = np.concatenate([res.results[c]["o"] for c in range(N_CORES)], axis=1)
    return np.ascontiguousarray(out[:, :L, :]), res


def kernel(inputs: np.ndarray, masks: np.ndarray, W: np.ndarray) -> np.ndarray:
    out, _ = _run(inputs, masks, W)
    return out
